# revision 4
# baseline (speedup 1.0000x reference)
"""Trainium2 Bass kernel for nn_DLT: batched 4-point DLT homography solve.

Math (per batch element, all elementwise over the batch in SoA layout):
  Classical projective-basis construction. With src points p0..p3 (x=a, y=b)
  and dst points q0..q3 (x=c, y=e), cyclic triples i,(j,k) = (i, i+1, i+2):
    u_i = b_j - b_k ; v_i = a_k - a_j ; w_i = a_j b_k - a_k b_j   (adj rows)
    lam_i = u_i a3 + v_i b3 + w_i
    Q_j = q_j - q3 ;  mu_i = cross(Q_j, Q_k)
    d_i = mu_i * lam_j * lam_k
    H'[r,c] = sum_i q_i[r] * d_i * adjS[i,c]   (q_i[2] = 1)
    H = H' / H'[2,2] ;  H[2,2] = 1
  This is algebraically the exact solve of the reference's 8x8 system.

Layout: per core B_pc = 65536 elements as [128 partitions x 512 free],
processed in NCHUNK chunks along the free dim. All compute is elementwise
fp32 on the Vector engine (fp32 tensor_tensor = 1x mode), batched into slab
instructions via strided (incl. step-0 broadcast) access patterns; ScalarE
builds the small duplication tiles that make cyclic index patterns affine.

Sharding: pure data-parallel over 8 NeuronCores (batch dim), zero comms.
"""

import os
import numpy as np

import concourse.bass as bass
import concourse.bacc as bacc
import concourse.tile as tile
from concourse import mybir
from concourse.bass_utils import run_bass_kernel_spmd

F32 = mybir.dt.float32

N_CORES = 8
B_FULL = 524288
B_PC = B_FULL // N_CORES  # 65536 per core
P = 128


def _ap(base, off, dims):
    """Strided AP over a pool tile. `base` = tile AP (partition dim first);
    `off` = element offset within a partition; `dims` = free-dim
    [step, count] pairs (outer -> inner), in elements."""
    return bass.AP(
        tensor=base.tensor,
        offset=base.offset + off,
        ap=[list(base.ap[0])] + [list(d) for d in dims],
    )


def build_kernel(nc, B=B_PC, F=None, nchunk=2):
    """Emit the kernel into `nc`. B must be P*F*nchunk."""
    if F is None:
        F = B // (P * nchunk)
    assert P * F * nchunk == B

    src = nc.dram_tensor("src_pt", [B, 4, 2], F32, kind="ExternalInput").ap()
    dst = nc.dram_tensor("dst_pt", [B, 4, 2], F32, kind="ExternalInput").ap()
    out = nc.dram_tensor("out", [B, 3, 3], F32, kind="ExternalOutput").ap()

    srcv = src.rearrange("(ch p f) n t -> ch p (f n t)", ch=nchunk, p=P)
    dstv = dst.rearrange("(ch p f) n t -> ch p (f n t)", ch=nchunk, p=P)
    outv = out.rearrange("(ch p f) r c -> ch p (f r c)", ch=nchunk, p=P)

    ts = mybir.AluOpType.subtract
    tm = mybir.AluOpType.mult
    ta = mybir.AluOpType.add

    with tile.TileContext(nc) as tc:
        with tc.tile_pool(name="raw", bufs=2) as rawp, \
             tc.tile_pool(name="work", bufs=1) as wp, \
             tc.tile_pool(name="outp", bufs=2) as outp:
            for ch in range(nchunk):
                rawS = rawp.tile([P, F, 8], F32, tag="rawS")
                rawD = rawp.tile([P, F, 8], F32, tag="rawD")
                nc.sync.dma_start(out=rawS, in_=srcv[ch])
                nc.sync.dma_start(out=rawD, in_=dstv[ch])

                ADJ = wp.tile([P, 9, F], F32, tag="ADJ")   # u(0:3) v(3:6) w(6:9)
                WS = wp.tile([P, 9, F], F32, tag="WS")     # scratch
                LAM = wp.tile([P, 5, F], F32, tag="LAM")   # lam(0:3) dup(3:5)
                AD = wp.tile([P, 5, F], F32, tag="AD")     # a0,a1,a2,a0',a1'
                BD = wp.tile([P, 5, F], F32, tag="BD")     # b0,b1,b2,b0',b1'
                QT = wp.tile([P, 10, F], F32, tag="QT")    # Q(0:6) dupQ0,Q1(6:10)
                QP = wp.tile([P, 6, F], F32, tag="QP")
                MU = wp.tile([P, 3, F], F32, tag="MU")
                M3 = wp.tile([P, 3, F], F32, tag="M3")
                D3 = wp.tile([P, 3, F], F32, tag="D3")
                G = wp.tile([P, 9, F], F32, tag="G")
                RP = wp.tile([P, 18, F], F32, tag="RP")
                SR = wp.tile([P, 6, F], F32, tag="SR")
                HP = wp.tile([P, 9, F], F32, tag="HP")
                RC = wp.tile([P, 2, F], F32, tag="RC")
                OT = outp.tile([P, F, 9], F32, tag="OT")

                def rawap(t, c, dims):
                    return _ap(t, c, dims + [[8, F]])

                def vap(t, slot, dims):
                    return _ap(t, slot * F, [[s * F, n] for s, n in dims] + [[1, F]])

                v = nc.vector
                sc = nc.scalar
                # dup tiles: AD = (a0,a1,a2,a0,a1), BD likewise (2 ACT copies each)
                sc.copy(out=vap(AD, 0, [[1, 3]]), in_=rawap(rawS, 0, [[2, 3]]))
                sc.copy(out=vap(AD, 3, [[1, 2]]), in_=vap(AD, 0, [[1, 2]]))
                sc.copy(out=vap(BD, 0, [[1, 3]]), in_=rawap(rawS, 1, [[2, 3]]))
                sc.copy(out=vap(BD, 3, [[1, 2]]), in_=vap(BD, 0, [[1, 2]]))
                # u_i = b_{i+1} - b_{i+2} -> ADJ(0:3)
                v.tensor_tensor(out=vap(ADJ, 0, [[1, 3]]),
                                in0=vap(BD, 1, [[1, 3]]),
                                in1=vap(BD, 2, [[1, 3]]), op=ts)
                # v_i = a_{i+2} - a_{i+1} -> ADJ(3:6)
                v.tensor_tensor(out=vap(ADJ, 3, [[1, 3]]),
                                in0=vap(AD, 2, [[1, 3]]),
                                in1=vap(AD, 1, [[1, 3]]), op=ts)
                # w+ = a_{i+1} b_{i+2} -> WS(0:3) ; w- = a_{i+2} b_{i+1} -> WS(3:6)
                v.tensor_tensor(out=vap(WS, 0, [[1, 3]]),
                                in0=vap(AD, 1, [[1, 3]]),
                                in1=vap(BD, 2, [[1, 3]]), op=tm)
                v.tensor_tensor(out=vap(WS, 3, [[1, 3]]),
                                in0=vap(AD, 2, [[1, 3]]),
                                in1=vap(BD, 1, [[1, 3]]), op=tm)
                # w = w+ - w- -> ADJ(6:9)
                v.tensor_tensor(out=vap(ADJ, 6, [[1, 3]]),
                                in0=vap(WS, 0, [[1, 3]]),
                                in1=vap(WS, 3, [[1, 3]]), op=ts)
                # (tu, tv) = (u,v) * (a3,b3) broadcast -> WS(0:6)
                v.tensor_tensor(out=_ap(WS, 0, [[3 * F, 2], [F, 3], [1, F]]),
                                in0=_ap(ADJ, 0, [[3 * F, 2], [F, 3], [1, F]]),
                                in1=_ap(rawS, 6, [[1, 2], [0, 3], [8, F]]), op=tm)
                # lam = (tu + tv) + w
                v.tensor_tensor(out=vap(WS, 6, [[1, 3]]),
                                in0=vap(WS, 0, [[1, 3]]),
                                in1=vap(WS, 3, [[1, 3]]), op=ta)
                v.tensor_tensor(out=vap(LAM, 0, [[1, 3]]),
                                in0=vap(WS, 6, [[1, 3]]),
                                in1=vap(ADJ, 6, [[1, 3]]), op=ta)
                # lam dup
                sc.copy(out=vap(LAM, 3, [[1, 2]]), in_=vap(LAM, 0, [[1, 2]]))
                # Q diffs: QT(0:6) = rawD(0..5) - (c3,e3) broadcast
                v.tensor_tensor(out=_ap(QT, 0, [[2 * F, 3], [F, 2], [1, F]]),
                                in0=_ap(rawD, 0, [[2, 3], [1, 2], [8, F]]),
                                in1=_ap(rawD, 6, [[0, 3], [1, 2], [8, F]]), op=ts)
                # dup Q0,Q1 -> QT(6:10)
                sc.copy(out=vap(QT, 6, [[1, 4]]), in_=vap(QT, 0, [[1, 4]]))
                # qprod: in0 = (Q_{i+1}x,Q_{i+1}y) pairs; in1 = (Q_{i+2}y,Q_{i+2}x)
                v.tensor_tensor(out=_ap(QP, 0, [[2 * F, 3], [F, 2], [1, F]]),
                                in0=_ap(QT, 2 * F, [[2 * F, 3], [F, 2], [1, F]]),
                                in1=_ap(QT, 5 * F, [[2 * F, 3], [-F, 2], [1, F]]), op=tm)
                # mu = evens - odds
                v.tensor_tensor(out=vap(MU, 0, [[1, 3]]),
                                in0=vap(QP, 0, [[2, 3]]),
                                in1=vap(QP, 1, [[2, 3]]), op=ts)
                # m_i = lam_{i+1}*lam_{i+2}
                v.tensor_tensor(out=vap(M3, 0, [[1, 3]]),
                                in0=vap(LAM, 1, [[1, 3]]),
                                in1=vap(LAM, 2, [[1, 3]]), op=tm)
                # d = mu * m
                v.tensor_tensor(out=vap(D3, 0, [[1, 3]]),
                                in0=vap(MU, 0, [[1, 3]]),
                                in1=vap(M3, 0, [[1, 3]]), op=tm)
                # g = ADJ * d (broadcast d over the 3 col-groups)
                v.tensor_tensor(out=_ap(G, 0, [[3 * F, 3], [F, 3], [1, F]]),
                                in0=_ap(ADJ, 0, [[3 * F, 3], [F, 3], [1, F]]),
                                in1=_ap(D3, 0, [[0, 3], [F, 3], [1, F]]), op=tm)
                # r0p = g * c_i ; r1p = g * e_i  (broadcast c/e over col-groups)
                v.tensor_tensor(out=_ap(RP, 0, [[3 * F, 3], [F, 3], [1, F]]),
                                in0=_ap(G, 0, [[3 * F, 3], [F, 3], [1, F]]),
                                in1=_ap(rawD, 0, [[0, 3], [2, 3], [8, F]]), op=tm)
                v.tensor_tensor(out=_ap(RP, 9 * F, [[3 * F, 3], [F, 3], [1, F]]),
                                in0=_ap(G, 0, [[3 * F, 3], [F, 3], [1, F]]),
                                in1=_ap(rawD, 1, [[0, 3], [2, 3], [8, F]]), op=tm)
                # suma = rp[3c+0] + rp[3c+1] (both rows) -> SR(0:6)
                v.tensor_tensor(out=_ap(SR, 0, [[3 * F, 2], [F, 3], [1, F]]),
                                in0=_ap(RP, 0, [[9 * F, 2], [3 * F, 3], [1, F]]),
                                in1=_ap(RP, F, [[9 * F, 2], [3 * F, 3], [1, F]]), op=ta)
                # sumb = suma + rp[3c+2] -> HP(0:6)
                v.tensor_tensor(out=_ap(HP, 0, [[3 * F, 2], [F, 3], [1, F]]),
                                in0=_ap(SR, 0, [[3 * F, 2], [F, 3], [1, F]]),
                                in1=_ap(RP, 2 * F, [[9 * F, 2], [3 * F, 3], [1, F]]), op=ta)
                # row2: (g0c + g1c) + g2c -> HP(6:9)
                v.tensor_tensor(out=vap(WS, 0, [[1, 3]]),
                                in0=vap(G, 0, [[3, 3]]),
                                in1=vap(G, 1, [[3, 3]]), op=ta)
                v.tensor_tensor(out=vap(HP, 6, [[1, 3]]),
                                in0=vap(WS, 0, [[1, 3]]),
                                in1=vap(G, 2, [[3, 3]]), op=ta)
                # rc = 1/H22 (approx + NR, ~2 ulp)
                v.reciprocal_approx_accurate(out=vap(RC, 0, [[1, 1]]),
                                             in_=vap(HP, 8, [[1, 1]]),
                                             scratch=vap(RC, 1, [[1, 1]]))
                # H = H' * rc -> OT strided (entry r3c at f*9 + (r*3+c))
                v.tensor_tensor(out=_ap(OT, 0, [[1, 8], [9, F]]),
                                in0=_ap(HP, 0, [[F, 8], [1, F]]),
                                in1=_ap(RC, 0, [[0, 8], [1, F]]), op=tm)
                # ones column
                nc.gpsimd.memset(_ap(OT, 8, [[9, F]]), 1.0)
                # store
                nc.sync.dma_start(out=outv[ch], in_=OT)
    return nc


def _build_full():
    nc = bacc.Bacc(
        "TRN2",
        target_bir_lowering=False,
        debug=False,
        enable_asserts=False,
    )
    build_kernel(nc, B=B_PC, nchunk=2)
    nc.compile()
    return nc


_NC_CACHE = None


def kernel(src_pt: np.ndarray, dst_pt: np.ndarray) -> np.ndarray:
    global _NC_CACHE
    src_pt = np.ascontiguousarray(np.asarray(src_pt), dtype=np.float32)
    dst_pt = np.ascontiguousarray(np.asarray(dst_pt), dtype=np.float32)
    assert src_pt.shape == (B_FULL, 4, 2), src_pt.shape

    if _NC_CACHE is None:
        _NC_CACHE = _build_full()
    nc = _NC_CACHE

    in_maps = []
    for k in range(N_CORES):
        sl = slice(k * B_PC, (k + 1) * B_PC)
        in_maps.append({"src_pt": src_pt[sl], "dst_pt": dst_pt[sl]})

    res = run_bass_kernel_spmd(
        nc,
        in_maps,
        core_ids=list(range(N_CORES)),
        trace=bool(int(os.environ.get("DLT_TRACE", "0"))),
    )
    out = np.empty((B_FULL, 3, 3), dtype=np.float32)
    for k in range(N_CORES):
        out[k * B_PC:(k + 1) * B_PC] = res.results[k]["out"]
    kernel.last_results = res
    return out


# revision 12
# speedup vs baseline: 1.0134x; 1.0134x over previous
"""Trainium2 Bass kernel for nn_DLT: batched 4-point DLT homography solve.

Math (per batch element, all elementwise over the batch in SoA layout):
  Classical projective-basis construction. With src points p0..p3 (x=a, y=b)
  and dst points q0..q3 (x=c, y=e), cyclic triples i,(j,k) = (i, i+1, i+2):
    u_i = b_j - b_k ; v_i = a_k - a_j ; w_i = a_j b_k - a_k b_j   (adj rows)
    lam_i = u_i a3 + v_i b3 + w_i
    Q_j = q_j - q3 ;  mu_i = cross(Q_j, Q_k)
    d_i = mu_i * lam_j * lam_k
    H'[r,c] = sum_i q_i[r] * d_i * adjS[i,c]   (q_i[2] = 1)
    H = H' / H'[2,2] ;  H[2,2] = 1
  This is algebraically the exact solve of the reference's 8x8 system.

Layout: per core B_pc = 65536 elements as [128 partitions x 512 free],
processed in NCHUNK chunks along the free dim. All compute is elementwise
fp32 on the Vector engine (fp32 tensor_tensor = 1x mode), batched into slab
instructions via strided (incl. step-0 broadcast) access patterns; ScalarE
builds the small duplication tiles that make cyclic index patterns affine.

Sharding: pure data-parallel over 8 NeuronCores (batch dim), zero comms.
"""

import os
import numpy as np

import concourse.bass as bass
import concourse.bacc as bacc
import concourse.tile as tile
from concourse import mybir
from concourse.bass_utils import run_bass_kernel_spmd

F32 = mybir.dt.float32

N_CORES = 8
B_FULL = 524288
B_PC = B_FULL // N_CORES  # 65536 per core
P = 128


def _ap(base, off, dims):
    """Strided AP over a pool tile. `base` = tile AP (partition dim first);
    `off` = element offset within a partition; `dims` = free-dim
    [step, count] pairs (outer -> inner), in elements."""
    return bass.AP(
        tensor=base.tensor,
        offset=base.offset + off,
        ap=[list(base.ap[0])] + [list(d) for d in dims],
    )


def build_kernel(nc, B=B_PC, F=None, nchunk=2):
    """Emit the kernel into `nc`. B must be P*F*nchunk."""
    if F is None:
        F = B // (P * nchunk)
    assert P * F * nchunk == B

    src = nc.dram_tensor("src_pt", [B, 4, 2], F32, kind="ExternalInput").ap()
    dst = nc.dram_tensor("dst_pt", [B, 4, 2], F32, kind="ExternalInput").ap()
    out = nc.dram_tensor("out", [B, 3, 3], F32, kind="ExternalOutput").ap()

    srcv = src.rearrange("(ch p f) n t -> ch p (f n t)", ch=nchunk, p=P)
    dstv = dst.rearrange("(ch p f) n t -> ch p (f n t)", ch=nchunk, p=P)
    outv = out.rearrange("(ch p f) r c -> ch p (f r c)", ch=nchunk, p=P)

    ts = mybir.AluOpType.subtract
    tm = mybir.AluOpType.mult
    ta = mybir.AluOpType.add

    with tile.TileContext(nc) as tc:
        with tc.tile_pool(name="raw", bufs=2) as rawp, \
             tc.tile_pool(name="work", bufs=1) as wp, \
             tc.tile_pool(name="outp", bufs=2) as outp:
            for ch in range(nchunk):
                rawS = rawp.tile([P, F, 8], F32, tag="rawS")
                rawD = rawp.tile([P, F, 8], F32, tag="rawD")
                nc.sync.dma_start(out=rawS, in_=srcv[ch])
                nc.sync.dma_start(out=rawD, in_=dstv[ch])

                ADJ = wp.tile([P, 9, F], F32, tag="ADJ")   # u(0:3) v(3:6) w(6:9)
                WS = wp.tile([P, 9, F], F32, tag="WS")     # scratch
                LAM = wp.tile([P, 5, F], F32, tag="LAM")   # lam(0:3) dup(3:5)
                # T10: a0,a1,a2,a0',a1' (0:5) | b0,b1,b2,b0',b1' (5:10) —
                # adjacent a/b blocks make the u+v and w+/w- slabs affine.
                T10 = wp.tile([P, 10, F], F32, tag="T10")
                QT = wp.tile([P, 10, F], F32, tag="QT")    # Q(0:6) dupQ0,Q1(6:10)
                QP = wp.tile([P, 6, F], F32, tag="QP")
                MU = wp.tile([P, 3, F], F32, tag="MU")
                M3 = wp.tile([P, 3, F], F32, tag="M3")
                D3 = wp.tile([P, 3, F], F32, tag="D3")
                G = wp.tile([P, 9, F], F32, tag="G")
                RP = wp.tile([P, 18, F], F32, tag="RP")
                SR = wp.tile([P, 6, F], F32, tag="SR")
                HP = wp.tile([P, 9, F], F32, tag="HP")
                RC = wp.tile([P, 2, F], F32, tag="RC")
                OT = outp.tile([P, F, 9], F32, tag="OT")

                def rawap(t, c, dims):
                    return _ap(t, c, dims + [[8, F]])

                def vap(t, slot, dims):
                    return _ap(t, slot * F, [[s * F, n] for s, n in dims] + [[1, F]])

                v = nc.vector
                sc = nc.scalar
                # T10 dup blocks — 4 independent ACT copies, all from rawS
                sc.copy(out=vap(T10, 0, [[1, 3]]), in_=rawap(rawS, 0, [[2, 3]]))
                sc.copy(out=vap(T10, 3, [[1, 2]]), in_=rawap(rawS, 0, [[2, 2]]))
                sc.copy(out=vap(T10, 5, [[1, 3]]), in_=rawap(rawS, 1, [[2, 3]]))
                sc.copy(out=vap(T10, 8, [[1, 2]]), in_=rawap(rawS, 1, [[2, 2]]))
                # u_i = b_{i+1} - b_{i+2} -> ADJ(0:3) ;
                # v_i = a_{i+2} - a_{i+1} -> ADJ(3:6)   (one slab instr)
                v.tensor_tensor(out=_ap(ADJ, 0, [[3 * F, 2], [F, 3], [1, F]]),
                                in0=_ap(T10, 6 * F, [[-4 * F, 2], [F, 3], [1, F]]),
                                in1=_ap(T10, 7 * F, [[-6 * F, 2], [F, 3], [1, F]]), op=ts)
                # w+ = a_{i+1} b_{i+2} -> WS(0:3) ; w- = a_{i+2} b_{i+1} -> WS(3:6)
                v.tensor_tensor(out=_ap(WS, 0, [[3 * F, 2], [F, 3], [1, F]]),
                                in0=_ap(T10, F, [[F, 2], [F, 3], [1, F]]),
                                in1=_ap(T10, 7 * F, [[-F, 2], [F, 3], [1, F]]), op=tm)
                # w = w+ - w- -> ADJ(6:9)
                v.tensor_tensor(out=vap(ADJ, 6, [[1, 3]]),
                                in0=vap(WS, 0, [[1, 3]]),
                                in1=vap(WS, 3, [[1, 3]]), op=ts)
                # (tu, tv) = (u,v) * (a3,b3) broadcast -> WS(0:6)
                v.tensor_tensor(out=_ap(WS, 0, [[3 * F, 2], [F, 3], [1, F]]),
                                in0=_ap(ADJ, 0, [[3 * F, 2], [F, 3], [1, F]]),
                                in1=_ap(rawS, 6, [[1, 2], [0, 3], [8, F]]), op=tm)
                # lam = (tu + tv) + w
                v.tensor_tensor(out=vap(WS, 6, [[1, 3]]),
                                in0=vap(WS, 0, [[1, 3]]),
                                in1=vap(WS, 3, [[1, 3]]), op=ta)
                v.tensor_tensor(out=vap(LAM, 0, [[1, 3]]),
                                in0=vap(WS, 6, [[1, 3]]),
                                in1=vap(ADJ, 6, [[1, 3]]), op=ta)
                # lam dup
                sc.copy(out=vap(LAM, 3, [[1, 2]]), in_=vap(LAM, 0, [[1, 2]]))
                # Q diffs: QT(0:6) = rawD(0..5) - (c3,e3) broadcast
                v.tensor_tensor(out=_ap(QT, 0, [[2 * F, 3], [F, 2], [1, F]]),
                                in0=_ap(rawD, 0, [[2, 3], [1, 2], [8, F]]),
                                in1=_ap(rawD, 6, [[0, 3], [1, 2], [8, F]]), op=ts)
                # dup Q0,Q1 -> QT(6:10)
                sc.copy(out=vap(QT, 6, [[1, 4]]), in_=vap(QT, 0, [[1, 4]]))
                # qprod: in0 = (Q_{i+1}x,Q_{i+1}y) pairs; in1 = (Q_{i+2}y,Q_{i+2}x)
                v.tensor_tensor(out=_ap(QP, 0, [[2 * F, 3], [F, 2], [1, F]]),
                                in0=_ap(QT, 2 * F, [[2 * F, 3], [F, 2], [1, F]]),
                                in1=_ap(QT, 5 * F, [[2 * F, 3], [-F, 2], [1, F]]), op=tm)
                # mu = evens - odds
                v.tensor_tensor(out=vap(MU, 0, [[1, 3]]),
                                in0=vap(QP, 0, [[2, 3]]),
                                in1=vap(QP, 1, [[2, 3]]), op=ts)
                # m_i = lam_{i+1}*lam_{i+2}
                v.tensor_tensor(out=vap(M3, 0, [[1, 3]]),
                                in0=vap(LAM, 1, [[1, 3]]),
                                in1=vap(LAM, 2, [[1, 3]]), op=tm)
                # d = mu * m
                v.tensor_tensor(out=vap(D3, 0, [[1, 3]]),
                                in0=vap(MU, 0, [[1, 3]]),
                                in1=vap(M3, 0, [[1, 3]]), op=tm)
                # g = ADJ * d (broadcast d over the 3 col-groups)
                v.tensor_tensor(out=_ap(G, 0, [[3 * F, 3], [F, 3], [1, F]]),
                                in0=_ap(ADJ, 0, [[3 * F, 3], [F, 3], [1, F]]),
                                in1=_ap(D3, 0, [[0, 3], [F, 3], [1, F]]), op=tm)
                # r0p = g * c_i ; r1p = g * e_i  (broadcast c/e over col-groups)
                v.tensor_tensor(out=_ap(RP, 0, [[3 * F, 3], [F, 3], [1, F]]),
                                in0=_ap(G, 0, [[3 * F, 3], [F, 3], [1, F]]),
                                in1=_ap(rawD, 0, [[0, 3], [2, 3], [8, F]]), op=tm)
                v.tensor_tensor(out=_ap(RP, 9 * F, [[3 * F, 3], [F, 3], [1, F]]),
                                in0=_ap(G, 0, [[3 * F, 3], [F, 3], [1, F]]),
                                in1=_ap(rawD, 1, [[0, 3], [2, 3], [8, F]]), op=tm)
                # suma = rp[3c+0] + rp[3c+1] (both rows) -> SR(0:6)
                v.tensor_tensor(out=_ap(SR, 0, [[3 * F, 2], [F, 3], [1, F]]),
                                in0=_ap(RP, 0, [[9 * F, 2], [3 * F, 3], [1, F]]),
                                in1=_ap(RP, F, [[9 * F, 2], [3 * F, 3], [1, F]]), op=ta)
                # sumb = suma + rp[3c+2] -> HP(0:6)
                v.tensor_tensor(out=_ap(HP, 0, [[3 * F, 2], [F, 3], [1, F]]),
                                in0=_ap(SR, 0, [[3 * F, 2], [F, 3], [1, F]]),
                                in1=_ap(RP, 2 * F, [[9 * F, 2], [3 * F, 3], [1, F]]), op=ta)
                # row2: (g0c + g1c) + g2c -> HP(6:9)
                v.tensor_tensor(out=vap(WS, 0, [[1, 3]]),
                                in0=vap(G, 0, [[3, 3]]),
                                in1=vap(G, 1, [[3, 3]]), op=ta)
                v.tensor_tensor(out=vap(HP, 6, [[1, 3]]),
                                in0=vap(WS, 0, [[1, 3]]),
                                in1=vap(G, 2, [[3, 3]]), op=ta)
                # rc = 1/H22 (~51 ulp ~ 6e-6 rel — negligible vs the ~3e-4
                # p999 of the solve itself; single custom-DVE instr)
                v.reciprocal_approx_fast(out=vap(RC, 0, [[1, 1]]),
                                         in_=vap(HP, 8, [[1, 1]]))
                # H = H' * rc -> OT strided (entry r3c at f*9 + (r*3+c))
                v.tensor_tensor(out=_ap(OT, 0, [[1, 8], [9, F]]),
                                in0=_ap(HP, 0, [[F, 8], [1, F]]),
                                in1=_ap(RC, 0, [[0, 8], [1, F]]), op=tm)
                # ones column on ACT (0*x + 1; gpsimd memset would grab the
                # SBUF port pair it shares with the Vector engine)
                sc.activation(_ap(OT, 8, [[9, F]]), _ap(rawD, 0, [[8, F]]),
                              mybir.ActivationFunctionType.Copy,
                              bias=1.0, scale=0.0)
                # store
                nc.sync.dma_start(out=outv[ch], in_=OT)
    return nc


def _build_full():
    nc = bacc.Bacc(
        "TRN2",
        target_bir_lowering=False,
        debug=False,
        enable_asserts=False,
    )
    build_kernel(nc, B=B_PC, nchunk=2)
    nc.compile()
    return nc


_NC_CACHE = None


def kernel(src_pt: np.ndarray, dst_pt: np.ndarray) -> np.ndarray:
    global _NC_CACHE
    src_pt = np.ascontiguousarray(np.asarray(src_pt), dtype=np.float32)
    dst_pt = np.ascontiguousarray(np.asarray(dst_pt), dtype=np.float32)
    assert src_pt.shape == (B_FULL, 4, 2), src_pt.shape

    if _NC_CACHE is None:
        _NC_CACHE = _build_full()
    nc = _NC_CACHE

    in_maps = []
    for k in range(N_CORES):
        sl = slice(k * B_PC, (k + 1) * B_PC)
        in_maps.append({"src_pt": src_pt[sl], "dst_pt": dst_pt[sl]})

    res = run_bass_kernel_spmd(
        nc,
        in_maps,
        core_ids=list(range(N_CORES)),
        trace=bool(int(os.environ.get("DLT_TRACE", "0"))),
    )
    out = np.empty((B_FULL, 3, 3), dtype=np.float32)
    for k in range(N_CORES):
        out[k * B_PC:(k + 1) * B_PC] = res.results[k]["out"]
    kernel.last_results = res
    return out


# revision 17
# speedup vs baseline: 1.0746x; 1.0604x over previous
"""Trainium2 Bass kernel for nn_DLT: batched 4-point DLT homography solve.

Math (per batch element, all elementwise over the batch in SoA layout):
  Classical projective-basis construction. With src points p0..p3 (x=a, y=b)
  and dst points q0..q3 (x=c, y=e), cyclic triples i,(j,k) = (i, i+1, i+2):
    u_i = b_j - b_k ; v_i = a_k - a_j ; w_i = a_j b_k - a_k b_j   (adj rows)
    lam_i = u_i a3 + v_i b3 + w_i
    Q_j = q_j - q3 ;  mu_i = cross(Q_j, Q_k)
    d_i = mu_i * lam_j * lam_k
    H'[r,c] = sum_i q_i[r] * d_i * adjS[i,c]   (q_i[2] = 1)
    H = H' / H'[2,2] ;  H[2,2] = 1
  This is algebraically the exact solve of the reference's 8x8 system.

Layout: per core B_pc = 65536 elements as [128 partitions x 512 free],
processed in NCHUNK chunks along the free dim. All compute is elementwise
fp32 on the Vector engine (fp32 tensor_tensor = 1x mode), batched into slab
instructions via strided (incl. step-0 broadcast) access patterns; ScalarE
builds the small duplication tiles that make cyclic index patterns affine.

Sharding: pure data-parallel over 8 NeuronCores (batch dim), zero comms.
"""

import os
import numpy as np

import concourse.bass as bass
import concourse.bacc as bacc
import concourse.tile as tile
from concourse import mybir
from concourse.bass_utils import run_bass_kernel_spmd

F32 = mybir.dt.float32

N_CORES = 8
B_FULL = 524288
B_PC = B_FULL // N_CORES  # 65536 per core
P = 128


def _ap(base, off, dims):
    """Strided AP over a pool tile. `base` = tile AP (partition dim first);
    `off` = element offset within a partition; `dims` = free-dim
    [step, count] pairs (outer -> inner), in elements."""
    return bass.AP(
        tensor=base.tensor,
        offset=base.offset + off,
        ap=[list(base.ap[0])] + [list(d) for d in dims],
    )


def build_kernel(nc, B=B_PC, F=None, nchunk=2):
    """Emit the kernel into `nc`. B must be P*F*nchunk."""
    if F is None:
        F = B // (P * nchunk)
    assert P * F * nchunk == B

    src = nc.dram_tensor("src_pt", [B, 4, 2], F32, kind="ExternalInput").ap()
    dst = nc.dram_tensor("dst_pt", [B, 4, 2], F32, kind="ExternalInput").ap()
    out = nc.dram_tensor("out", [B, 3, 3], F32, kind="ExternalOutput").ap()

    srcv = src.rearrange("(ch p f) n t -> ch p (f n t)", ch=nchunk, p=P)
    dstv = dst.rearrange("(ch p f) n t -> ch p (f n t)", ch=nchunk, p=P)
    outv = out.rearrange("(ch p f) r c -> ch p (f r c)", ch=nchunk, p=P)

    ts = mybir.AluOpType.subtract
    tm = mybir.AluOpType.mult
    ta = mybir.AluOpType.add

    with tile.TileContext(nc) as tc:
        with tc.tile_pool(name="raw", bufs=2) as rawp, \
             tc.tile_pool(name="work", bufs=1) as wp, \
             tc.tile_pool(name="outp", bufs=2) as outp:
            for ch in range(nchunk):
                rawS = rawp.tile([P, F, 8], F32, tag="rawS")
                rawD = rawp.tile([P, F, 8], F32, tag="rawD")
                nc.sync.dma_start(out=rawS, in_=srcv[ch])
                nc.sync.dma_start(out=rawD, in_=dstv[ch])

                ADJ = wp.tile([P, 9, F], F32, tag="ADJ")   # u(0:3) v(3:6) w(6:9)
                WS = wp.tile([P, 9, F], F32, tag="WS")     # scratch
                LAM = wp.tile([P, 5, F], F32, tag="LAM")   # lam(0:3) dup(3:5)
                # T10: a0,a1,a2,a0',a1' (0:5) | b0,b1,b2,b0',b1' (5:10) —
                # adjacent a/b blocks make the u+v and w+/w- slabs affine.
                # (chunk 0 reads rawS directly and skips it)
                if ch > 0:
                    T10 = wp.tile([P, 10, F], F32, tag="T10", name="T10")
                else:
                    T10 = None
                QT = wp.tile([P, 10, F], F32, tag="QT")    # Q(0:6) dupQ0,Q1(6:10)
                QP = wp.tile([P, 6, F], F32, tag="QP")
                MU = wp.tile([P, 3, F], F32, tag="MU")
                M3 = wp.tile([P, 3, F], F32, tag="M3")
                D3 = wp.tile([P, 3, F], F32, tag="D3")
                G = wp.tile([P, 9, F], F32, tag="G")
                RP = wp.tile([P, 18, F], F32, tag="RP")
                SR = wp.tile([P, 6, F], F32, tag="SR")
                HP = wp.tile([P, 9, F], F32, tag="HP")
                RC = wp.tile([P, 2, F], F32, tag="RC")
                OT = outp.tile([P, F, 9], F32, tag="OT")

                def rawap(t, c, dims):
                    return _ap(t, c, dims + [[8, F]])

                def vap(t, slot, dims):
                    return _ap(t, slot * F, [[s * F, n] for s, n in dims] + [[1, F]])

                v = nc.vector
                sc = nc.scalar
                if ch == 0:
                    # First chunk: read rawS directly (4 small extra instrs)
                    # instead of waiting on the ACT dup-tile chain — cuts the
                    # kernel-head latency where DVE would idle.
                    v.tensor_tensor(out=vap(ADJ, 0, [[1, 2]]),
                                    in0=rawap(rawS, 3, [[2, 2]]),
                                    in1=rawap(rawS, 5, [[-4, 2]]), op=ts)
                    v.tensor_tensor(out=vap(ADJ, 2, [[1, 1]]),
                                    in0=rawap(rawS, 1, [[2, 1]]),
                                    in1=rawap(rawS, 3, [[2, 1]]), op=ts)
                    v.tensor_tensor(out=vap(ADJ, 3, [[1, 2]]),
                                    in0=rawap(rawS, 4, [[-4, 2]]),
                                    in1=rawap(rawS, 2, [[2, 2]]), op=ts)
                    v.tensor_tensor(out=vap(ADJ, 5, [[1, 1]]),
                                    in0=rawap(rawS, 2, [[2, 1]]),
                                    in1=rawap(rawS, 0, [[2, 1]]), op=ts)
                    v.tensor_tensor(out=vap(WS, 0, [[1, 2]]),
                                    in0=rawap(rawS, 2, [[2, 2]]),
                                    in1=rawap(rawS, 5, [[-4, 2]]), op=tm)
                    v.tensor_tensor(out=vap(WS, 2, [[1, 1]]),
                                    in0=rawap(rawS, 0, [[2, 1]]),
                                    in1=rawap(rawS, 3, [[2, 1]]), op=tm)
                    v.tensor_tensor(out=vap(WS, 3, [[1, 2]]),
                                    in0=rawap(rawS, 4, [[-4, 2]]),
                                    in1=rawap(rawS, 3, [[2, 2]]), op=tm)
                    v.tensor_tensor(out=vap(WS, 5, [[1, 1]]),
                                    in0=rawap(rawS, 2, [[2, 1]]),
                                    in1=rawap(rawS, 1, [[2, 1]]), op=tm)
                else:
                    # T10 dup blocks — 4 independent ACT copies, all from rawS
                    sc.copy(out=vap(T10, 0, [[1, 3]]), in_=rawap(rawS, 0, [[2, 3]]))
                    sc.copy(out=vap(T10, 3, [[1, 2]]), in_=rawap(rawS, 0, [[2, 2]]))
                    sc.copy(out=vap(T10, 5, [[1, 3]]), in_=rawap(rawS, 1, [[2, 3]]))
                    sc.copy(out=vap(T10, 8, [[1, 2]]), in_=rawap(rawS, 1, [[2, 2]]))
                    # u_i = b_{i+1} - b_{i+2} ; v_i = a_{i+2} - a_{i+1}
                    v.tensor_tensor(out=_ap(ADJ, 0, [[3 * F, 2], [F, 3], [1, F]]),
                                    in0=_ap(T10, 6 * F, [[-4 * F, 2], [F, 3], [1, F]]),
                                    in1=_ap(T10, 7 * F, [[-6 * F, 2], [F, 3], [1, F]]), op=ts)
                    # w+ = a_{i+1} b_{i+2} ; w- = a_{i+2} b_{i+1}
                    v.tensor_tensor(out=_ap(WS, 0, [[3 * F, 2], [F, 3], [1, F]]),
                                    in0=_ap(T10, F, [[F, 2], [F, 3], [1, F]]),
                                    in1=_ap(T10, 7 * F, [[-F, 2], [F, 3], [1, F]]), op=tm)
                # w = w+ - w- -> ADJ(6:9)
                v.tensor_tensor(out=vap(ADJ, 6, [[1, 3]]),
                                in0=vap(WS, 0, [[1, 3]]),
                                in1=vap(WS, 3, [[1, 3]]), op=ts)
                # (tu, tv) = (u,v) * (a3,b3) broadcast -> WS(0:6)
                v.tensor_tensor(out=_ap(WS, 0, [[3 * F, 2], [F, 3], [1, F]]),
                                in0=_ap(ADJ, 0, [[3 * F, 2], [F, 3], [1, F]]),
                                in1=_ap(rawS, 6, [[1, 2], [0, 3], [8, F]]), op=tm)
                # lam = (tu + tv) + w
                v.tensor_tensor(out=vap(WS, 6, [[1, 3]]),
                                in0=vap(WS, 0, [[1, 3]]),
                                in1=vap(WS, 3, [[1, 3]]), op=ta)
                v.tensor_tensor(out=vap(LAM, 0, [[1, 3]]),
                                in0=vap(WS, 6, [[1, 3]]),
                                in1=vap(ADJ, 6, [[1, 3]]), op=ta)
                # lam dup
                sc.copy(out=vap(LAM, 3, [[1, 2]]), in_=vap(LAM, 0, [[1, 2]]))
                # Q diffs: QT(0:6) = rawD(0..5) - (c3,e3) broadcast
                v.tensor_tensor(out=_ap(QT, 0, [[2 * F, 3], [F, 2], [1, F]]),
                                in0=_ap(rawD, 0, [[2, 3], [1, 2], [8, F]]),
                                in1=_ap(rawD, 6, [[0, 3], [1, 2], [8, F]]), op=ts)
                # dup Q0,Q1 -> QT(6:10)
                sc.copy(out=vap(QT, 6, [[1, 4]]), in_=vap(QT, 0, [[1, 4]]))
                # qprod: in0 = (Q_{i+1}x,Q_{i+1}y) pairs; in1 = (Q_{i+2}y,Q_{i+2}x)
                v.tensor_tensor(out=_ap(QP, 0, [[2 * F, 3], [F, 2], [1, F]]),
                                in0=_ap(QT, 2 * F, [[2 * F, 3], [F, 2], [1, F]]),
                                in1=_ap(QT, 5 * F, [[2 * F, 3], [-F, 2], [1, F]]), op=tm)
                # mu = evens - odds
                v.tensor_tensor(out=vap(MU, 0, [[1, 3]]),
                                in0=vap(QP, 0, [[2, 3]]),
                                in1=vap(QP, 1, [[2, 3]]), op=ts)
                # m_i = lam_{i+1}*lam_{i+2}
                v.tensor_tensor(out=vap(M3, 0, [[1, 3]]),
                                in0=vap(LAM, 1, [[1, 3]]),
                                in1=vap(LAM, 2, [[1, 3]]), op=tm)
                # d = mu * m
                v.tensor_tensor(out=vap(D3, 0, [[1, 3]]),
                                in0=vap(MU, 0, [[1, 3]]),
                                in1=vap(M3, 0, [[1, 3]]), op=tm)
                # g = ADJ * d (broadcast d over the 3 col-groups)
                v.tensor_tensor(out=_ap(G, 0, [[3 * F, 3], [F, 3], [1, F]]),
                                in0=_ap(ADJ, 0, [[3 * F, 3], [F, 3], [1, F]]),
                                in1=_ap(D3, 0, [[0, 3], [F, 3], [1, F]]), op=tm)
                # r0p = g * c_i ; r1p = g * e_i  (broadcast c/e over col-groups)
                v.tensor_tensor(out=_ap(RP, 0, [[3 * F, 3], [F, 3], [1, F]]),
                                in0=_ap(G, 0, [[3 * F, 3], [F, 3], [1, F]]),
                                in1=_ap(rawD, 0, [[0, 3], [2, 3], [8, F]]), op=tm)
                v.tensor_tensor(out=_ap(RP, 9 * F, [[3 * F, 3], [F, 3], [1, F]]),
                                in0=_ap(G, 0, [[3 * F, 3], [F, 3], [1, F]]),
                                in1=_ap(rawD, 1, [[0, 3], [2, 3], [8, F]]), op=tm)
                # suma = rp[3c+0] + rp[3c+1] (both rows) -> SR(0:6)
                v.tensor_tensor(out=_ap(SR, 0, [[3 * F, 2], [F, 3], [1, F]]),
                                in0=_ap(RP, 0, [[9 * F, 2], [3 * F, 3], [1, F]]),
                                in1=_ap(RP, F, [[9 * F, 2], [3 * F, 3], [1, F]]), op=ta)
                # sumb = suma + rp[3c+2] -> HP(0:6)
                v.tensor_tensor(out=_ap(HP, 0, [[3 * F, 2], [F, 3], [1, F]]),
                                in0=_ap(SR, 0, [[3 * F, 2], [F, 3], [1, F]]),
                                in1=_ap(RP, 2 * F, [[9 * F, 2], [3 * F, 3], [1, F]]), op=ta)
                # row2: (g0c + g1c) + g2c -> HP(6:9)
                v.tensor_tensor(out=vap(WS, 0, [[1, 3]]),
                                in0=vap(G, 0, [[3, 3]]),
                                in1=vap(G, 1, [[3, 3]]), op=ta)
                v.tensor_tensor(out=vap(HP, 6, [[1, 3]]),
                                in0=vap(WS, 0, [[1, 3]]),
                                in1=vap(G, 2, [[3, 3]]), op=ta)
                # rc = 1/H22 (~51 ulp ~ 6e-6 rel — negligible vs the ~3e-4
                # p999 of the solve itself; single custom-DVE instr)
                v.reciprocal_approx_fast(out=vap(RC, 0, [[1, 1]]),
                                         in_=vap(HP, 8, [[1, 1]]))
                # ones column on ACT (0*x + 1; gpsimd memset would grab the
                # SBUF port pair it shares with the Vector engine)
                sc.activation(_ap(OT, 8, [[9, F]]), _ap(rawD, 0, [[8, F]]),
                              mybir.ActivationFunctionType.Copy,
                              bias=1.0, scale=0.0)
                # H = H' * rc -> OT strided (entry r3c at f*9 + (r*3+c)).
                # Last chunk: two halves so the store overlaps the second.
                nhalf = 4 if ch == nchunk - 1 else 1
                Fh = F // nhalf
                for h in range(nhalf):
                    v.tensor_tensor(out=_ap(OT, 9 * h * Fh, [[1, 8], [9, Fh]]),
                                    in0=_ap(HP, h * Fh, [[F, 8], [1, Fh]]),
                                    in1=_ap(RC, h * Fh, [[0, 8], [1, Fh]]), op=tm)
                    nc.sync.dma_start(out=outv[ch][:, h * Fh * 9:(h + 1) * Fh * 9],
                                      in_=OT[:, h * Fh:(h + 1) * Fh, :])
    return nc


def _build_full():
    nc = bacc.Bacc(
        "TRN2",
        target_bir_lowering=False,
        debug=False,
        enable_asserts=False,
    )
    build_kernel(nc, B=B_PC, nchunk=2)
    nc.compile()
    return nc


_NC_CACHE = None
_EXEC_CACHE = None


def _get_exec():
    """Build the 8-core sharded executable once and cache it, so repeated
    kernel() calls don't re-trace/re-compile through XLA (a fresh
    run_bass_kernel_spmd call builds a new jit closure every time)."""
    global _NC_CACHE, _EXEC_CACHE
    if _EXEC_CACHE is not None:
        return _EXEC_CACHE
    import jax
    from jax.sharding import Mesh, PartitionSpec
    from jax.experimental.shard_map import shard_map
    from concourse import bass2jax

    if _NC_CACHE is None:
        _NC_CACHE = _build_full()
    nc = _NC_CACHE
    bass2jax.install_neuronx_cc_hook()

    partition_name = (nc.partition_id_tensor.name
                      if nc.partition_id_tensor else None)
    in_names, out_names, out_avals, zero_outs = [], [], [], []
    for alloc in nc.m.functions[0].allocations:
        if not isinstance(alloc, mybir.MemoryLocationSet):
            continue
        name = alloc.memorylocations[0].name
        if alloc.kind == "ExternalInput":
            if name != partition_name:
                in_names.append(name)
        elif alloc.kind == "ExternalOutput":
            shape = tuple(alloc.tensor_shape)
            dtype = mybir.dt.np(alloc.dtype)
            out_names.append(name)
            out_avals.append(jax.core.ShapedArray(shape, dtype))
            zero_outs.append(np.zeros((N_CORES * shape[0], *shape[1:]), dtype))
    n_params = len(in_names)
    all_in_names = list(in_names) + list(out_names)
    if partition_name is not None:
        all_in_names.append(partition_name)

    def _body(*args):
        operands = list(args)
        if partition_name is not None:
            operands.append(bass2jax.partition_id_tensor())
        outs = bass2jax._bass_exec_p.bind(
            *operands,
            out_avals=tuple(out_avals),
            in_names=tuple(all_in_names),
            out_names=tuple(out_names),
            lowering_input_output_aliases=(),
            sim_require_finite=True,
            sim_require_nnan=True,
            nc=nc,
        )
        return tuple(outs)

    devices = jax.devices()[:N_CORES]
    assert len(devices) == N_CORES, f"need {N_CORES} devices, have {len(devices)}"
    mesh = Mesh(np.asarray(devices), ("core",))
    in_specs = (PartitionSpec("core"),) * (n_params + len(out_names))
    out_specs = (PartitionSpec("core"),) * len(out_names)
    fn = jax.jit(shard_map(_body, mesh=mesh, in_specs=in_specs,
                           out_specs=out_specs, check_rep=False))
    _EXEC_CACHE = (fn, in_names, zero_outs)
    return _EXEC_CACHE


def kernel(src_pt: np.ndarray, dst_pt: np.ndarray) -> np.ndarray:
    global _NC_CACHE
    src_pt = np.ascontiguousarray(np.asarray(src_pt), dtype=np.float32)
    dst_pt = np.ascontiguousarray(np.asarray(dst_pt), dtype=np.float32)
    assert src_pt.shape == (B_FULL, 4, 2), src_pt.shape

    try:
        import jax
        fn, in_names, zero_outs = _get_exec()
        named = {"src_pt": src_pt, "dst_pt": dst_pt}
        args = [named[n] for n in in_names] + list(zero_outs)
        outs = fn(*args)
        jax.block_until_ready(outs)
        return np.asarray(outs[0]).reshape(B_FULL, 3, 3)
    except Exception:
        # Fallback: the stock multi-core runner (fresh jit per call).
        if _NC_CACHE is None:
            _NC_CACHE = _build_full()
        in_maps = []
        for k in range(N_CORES):
            sl = slice(k * B_PC, (k + 1) * B_PC)
            in_maps.append({"src_pt": src_pt[sl], "dst_pt": dst_pt[sl]})
        res = run_bass_kernel_spmd(
            _NC_CACHE,
            in_maps,
            core_ids=list(range(N_CORES)),
            trace=bool(int(os.environ.get("DLT_TRACE", "0"))),
        )
        out = np.empty((B_FULL, 3, 3), dtype=np.float32)
        for k in range(N_CORES):
            out[k * B_PC:(k + 1) * B_PC] = res.results[k]["out"]
        kernel.last_results = res
        return out
